# revision 4
# baseline (speedup 1.0000x reference)
"""Trainium2 Bass kernel for nn_AveragePoolingClassLoss (v2: bf16 + int16 codes).

Reference computation (per image):
  pred = softmax(logits[:, :5], axis=1)            # drop background ch 5
  idx  = argmax_c pred                             # per-pixel class
  s_c  = sum of pred[c] over pixels with idx == c  # == sum of per-pixel max prob
  n_c  = count of pixels with idx == c
  agg  = s_c / n_c (0 if n_c == 0)
  loss = BCE(agg, class_gt), mean over (image, class), log clamp -100

v2 strategy:
  * Inputs are cast to bf16 on the host: halves the HBM DMA (the memory
    roofline) with negligible effect on the final scalar (verified 7e-5).
  * exp() is replaced by the Schraudolph trick: k_c = int16(A*l + B) with
    A = 128/ln2; the int16 bit pattern reinterpreted as bf16 IS approx
    exp(l_c) (piecewise-linear-in-mantissa).  One cheap 4x-mode
    tensor_scalar pass replaces five ACT exp passes per image.  The global
    bias B cancels exactly in m = e_max/T, and argmax on codes == argmax
    on logits (monotone).  End-to-end rel err vs fp32 reference: ~7e-5.
  * max tree + equality masks run on int16 codes (exact integer compare).
  * counts ride free on the mask passes (accum_out), sum_m on the m pass.
  * T = sum_c e_c and the masked sums S_c = sum_p m*g_c go to the PE
    (identity matmuls resp. lhsT=m chunk traces); 1/T via ACT Ln+Exp.
  * class 4 stats by subtraction (sum_m, HW totals).

Sharding: pure data parallel over the batch: 8 cores x 4 images.
Each core emits the partial BCE numerator sum over its 20 (image, class)
pairs; the host sums the 8 partials and scales.
"""

import numpy as np
import ml_dtypes
from contextlib import ExitStack

import concourse.bass as bass
import concourse.bacc as bacc
import concourse.mybir as mybir
import concourse.tile as tile
from concourse import bass_isa, masks
from concourse.bass_utils import run_bass_kernel_spmd

F32 = mybir.dt.float32
BF16 = mybir.dt.bfloat16
I16 = mybir.dt.int16
ALU = mybir.AluOpType
ACTF = mybir.ActivationFunctionType

N_CORES = 8
IMGS_PER_CORE = 4
N_CLASSES = 5
HW = 512 * 512           # pixels per image
P = 128                  # partitions
FD = HW // P             # 2048 free-dim elements per plane
NSTAT = 9                # per image: [S0..S3, sum_m, G0..G3]
LOG_CLAMP = -100.0

# Schraudolph bf16-exp code constants: int16(A*l + B) bitcast bf16 ~ exp(l)
SCHR_A = 128.0 / float(np.log(2.0))
SCHR_B = 16256.0 - 7.335


def _build_program(repeat: int = 1):
    nc = bacc.Bacc(
        "TRN2",
        target_bir_lowering=False,
        debug=False,
        enable_asserts=False,
        num_devices=N_CORES,
    )

    logits = nc.dram_tensor(
        "logits", [IMGS_PER_CORE, N_CLASSES, 512, 512], BF16, kind="ExternalInput"
    )
    gt = nc.dram_tensor("gt", [IMGS_PER_CORE, N_CLASSES], F32, kind="ExternalInput")
    partial = nc.dram_tensor("partial", [1, 1], F32, kind="ExternalOutput")

    with ExitStack() as ctx:
        tc = ctx.enter_context(tile.TileContext(nc))
        _kernel_body(ctx, tc, logits.ap(), gt.ap(), partial.ap(), repeat)

    nc.compile()
    return nc


def _kernel_body(ctx, tc, logits, gt, partial, repeat=1):
    nc = tc.nc

    lpool = ctx.enter_context(tc.tile_pool(name="planes", bufs=2))
    kpool = ctx.enter_context(tc.tile_pool(name="codes", bufs=2))
    wpool = ctx.enter_context(tc.tile_pool(name="work", bufs=2))
    spool = ctx.enter_context(tc.tile_pool(name="stats", bufs=2))
    tpool = ctx.enter_context(tc.tile_pool(name="tpsum", bufs=1, space="PSUM"))
    ppool = ctx.enter_context(tc.tile_pool(name="psumT", bufs=2, space="PSUM"))

    ident = spool.tile([P, P], BF16, tag="ident")
    masks.make_identity(nc, ident[:])

    pools = (lpool, kpool, wpool, tpool, ppool, ident)
    for rep in range(repeat):
        stats = spool.tile([P, IMGS_PER_CORE * NSTAT], F32, tag="stats")
        for i in range(IMGS_PER_CORE):
            _image_pass(tc, pools, stats, logits, i)

    allred = spool.tile([P, IMGS_PER_CORE * NSTAT], F32, tag="allred")
    nc.gpsimd.partition_all_reduce(
        allred[:], stats[:], channels=P, reduce_op=bass_isa.ReduceOp.add
    )

    _bce_tail(ctx, tc, allred, gt, partial)


def _image_pass(tc, pools, stats, logits, i):
    nc = tc.nc
    lpool, kpool, wpool, tpool, ppool, ident = pools
    sb = i * NSTAT
    CH = 512                     # psum/T chunk columns
    TC = 128                     # trace chunk columns

    # ---- DMA the 5 bf16 planes into one wide tile --------------------------
    L = lpool.tile([P, N_CLASSES * FD], BF16, tag="L")
    for c in range(N_CLASSES):
        src = logits[i, c].rearrange("(p a) b -> p (a b)", p=P)
        nc.sync.dma_start(out=L[:, c * FD:(c + 1) * FD], in_=src)

    # ---- Schraudolph codes: one wide 4x-mode pass (DVE) --------------------
    K = kpool.tile([P, N_CLASSES * FD], I16, tag="K")
    nc.vector.tensor_scalar(
        out=K[:], in0=L[:], scalar1=SCHR_A, scalar2=SCHR_B,
        op0=ALU.mult, op1=ALU.add,
    )
    K3 = K[:].rearrange("p (c f) -> p c f", c=N_CLASSES)
    Kb = K[:].bitcast(BF16)      # the same bits viewed as bf16 ~ exp(l)

    # ---- max tree on int16 codes ------------------------------------------
    # lvl1: [t01 | t23] in one pass on planes {0,2} vs {1,3}
    t2 = wpool.tile([P, 2 * FD], I16, tag="t2")
    t23d = t2[:].rearrange("p (c f) -> p c f", c=2)
    nc.vector.tensor_tensor(t23d, K3[:, 0:3:2], K3[:, 1:4:2], ALU.max)
    t03 = wpool.tile([P, FD], I16, tag="t03")
    nc.vector.tensor_tensor(t03[:], t2[:, 0:FD], t2[:, FD:2 * FD], ALU.max)
    kmax = wpool.tile([P, FD], I16, tag="kmax")
    nc.vector.tensor_tensor(kmax[:], t03[:], K3[:, 4], ALU.max)

    # ---- masks + counts: g_c = [k_c == kmax], count rides on accum --------
    G = wpool.tile([P, 4 * FD], BF16, tag="G")
    for c in range(4):
        nc.vector.scalar_tensor_tensor(
            out=G[:, c * FD:(c + 1) * FD], in0=K3[:, c], scalar=1.0, in1=kmax[:],
            op0=ALU.mult, op1=ALU.is_equal,
            accum_out=stats[:, sb + 5 + c: sb + 6 + c],
        )

    # ---- T = sum_c e_c per chunk on PE; r = exp(-ln T) on ACT -------------
    r = wpool.tile([P, FD], BF16, tag="r")
    for k in range(FD // CH):
        Tps = ppool.tile([P, CH], F32, tag="Tps")
        for c in range(N_CLASSES):
            nc.tensor.matmul(
                out=Tps[:],
                lhsT=ident[:],
                rhs=Kb[:, c * FD + k * CH: c * FD + (k + 1) * CH],
                start=(c == 0), stop=(c == N_CLASSES - 1),
            )
        lnT = wpool.tile([P, CH], F32, tag="lnT")
        nc.scalar.activation(lnT[:], Tps[:], ACTF.Ln)
        nc.scalar.activation(r[:, k * CH:(k + 1) * CH], lnT[:], ACTF.Exp, scale=-1.0)

    # ---- m = e_max * r (+ sum_m) ------------------------------------------
    kmaxb = kmax[:].bitcast(BF16)
    m = wpool.tile([P, FD], BF16, tag="m")
    nc.vector.scalar_tensor_tensor(
        out=m[:], in0=kmaxb, scalar=1.0, in1=r[:],
        op0=ALU.mult, op1=ALU.mult,
        accum_out=stats[:, sb + 4: sb + 5],
    )

    # ---- masked sums via PE traces: tp_c += m_chunk.T @ g_chunk -----------
    tps = []
    for c in range(4):
        tpc = tpool.tile([P, TC], F32, tag=f"tp{c}")
        tps.append(tpc)
    nk = FD // TC
    for k in range(nk):
        for c in range(4):
            nc.tensor.matmul(
                out=tps[c][:],
                lhsT=m[:, k * TC:(k + 1) * TC],
                rhs=G[:, c * FD + k * TC: c * FD + k * TC + TC],
                start=(k == 0), stop=(k == nk - 1),
            )
    # S_c = trace(tp_c): fused mult-by-identity + accum
    for c in range(4):
        dg = wpool.tile([P, TC], F32, tag="dg")
        nc.vector.scalar_tensor_tensor(
            out=dg[:], in0=tps[c][:], scalar=1.0, in1=ident[:],
            op0=ALU.mult, op1=ALU.mult,
            accum_out=stats[:, sb + c: sb + 1 + c],
        )


def _bce_tail(ctx, tc, allred, gt, partial):
    """Tiny per-core tail on partition 0: build per-(image,class) agg then BCE."""
    nc = tc.nc
    tpool = ctx.enter_context(tc.tile_pool(name="tail", bufs=1))
    NI, NC5 = IMGS_PER_CORE, N_CLASSES
    n20 = NI * NC5

    st = allred[0:1, :]                      # [1, 36]
    st3 = st.rearrange("p (i k) -> p i k", k=NSTAT)  # [1, 4, 9]

    ssum = tpool.tile([1, NI], F32, tag="ssum")
    gsum = tpool.tile([1, NI], F32, tag="gsum")
    nc.vector.reduce_sum(ssum[:], st3[:, :, 0:4], axis=mybir.AxisListType.X)
    nc.vector.reduce_sum(gsum[:], st3[:, :, 5:9], axis=mybir.AxisListType.X)

    A = tpool.tile([1, n20], F32, tag="A")
    C = tpool.tile([1, n20], F32, tag="C")
    A3 = A.rearrange("p (i c) -> p i c", c=NC5)
    C3 = C.rearrange("p (i c) -> p i c", c=NC5)
    nc.vector.tensor_copy(A3[:, :, 0:4], st3[:, :, 0:4])
    nc.vector.tensor_copy(C3[:, :, 0:4], st3[:, :, 5:9])
    nc.vector.tensor_tensor(A3[:, :, 4], st3[:, :, 4], ssum[:], ALU.subtract)
    nc.vector.tensor_scalar(
        out=C3[:, :, 4], in0=gsum[:], scalar1=-1.0, scalar2=float(HW),
        op0=ALU.mult, op1=ALU.add,
    )

    nc.vector.tensor_scalar_max(C[:], C[:], 1.0)
    rc = tpool.tile([1, n20], F32, tag="rc")
    nc.vector.reciprocal(rc[:], C[:])
    agg = tpool.tile([1, n20], F32, tag="agg")
    nc.vector.tensor_tensor(agg[:], A[:], rc[:], ALU.mult)

    logp = tpool.tile([1, n20], F32, tag="logp")
    q = tpool.tile([1, n20], F32, tag="q")
    logq = tpool.tile([1, n20], F32, tag="logq")
    nc.scalar.activation(logp[:], agg[:], ACTF.Ln)
    nc.vector.tensor_scalar_max(logp[:], logp[:], LOG_CLAMP)
    nc.vector.tensor_scalar(
        out=q[:], in0=agg[:], scalar1=-1.0, scalar2=1.0, op0=ALU.mult, op1=ALU.add
    )
    nc.scalar.activation(logq[:], q[:], ACTF.Ln)
    nc.vector.tensor_scalar_max(logq[:], logq[:], LOG_CLAMP)

    gtt = tpool.tile([1, n20], F32, tag="gtt")
    nc.sync.dma_start(out=gtt[:], in_=gt.rearrange("(o i) c -> o (i c)", o=1))
    t1 = tpool.tile([1, n20], F32, tag="t1")
    nc.vector.tensor_tensor(t1[:], gtt[:], logp[:], ALU.mult)
    gtc = tpool.tile([1, n20], F32, tag="gtc")
    nc.vector.tensor_scalar(
        out=gtc[:], in0=gtt[:], scalar1=-1.0, scalar2=1.0, op0=ALU.mult, op1=ALU.add
    )
    t2 = tpool.tile([1, n20], F32, tag="t2")
    nc.vector.tensor_tensor(t2[:], gtc[:], logq[:], ALU.mult)
    tsum = tpool.tile([1, n20], F32, tag="tsum")
    nc.vector.tensor_tensor(tsum[:], t1[:], t2[:], ALU.add)
    out = tpool.tile([1, 1], F32, tag="out")
    nc.vector.reduce_sum(out[:], tsum[:], axis=mybir.AxisListType.X)
    nc.sync.dma_start(out=partial[:], in_=out[:])


_NC_CACHE = {}


def _get_program(repeat: int = 1):
    if repeat not in _NC_CACHE:
        _NC_CACHE[repeat] = _build_program(repeat)
    return _NC_CACHE[repeat]


def make_in_maps(segmentation_logits: np.ndarray, class_gt: np.ndarray):
    seg16 = segmentation_logits[:, :N_CLASSES].astype(ml_dtypes.bfloat16)
    gt32 = np.ascontiguousarray(class_gt, dtype=np.float32)
    in_maps = []
    for core in range(N_CORES):
        lo = core * IMGS_PER_CORE
        hi = lo + IMGS_PER_CORE
        in_maps.append({
            "logits": np.ascontiguousarray(seg16[lo:hi]),
            "gt": np.ascontiguousarray(gt32[lo:hi]),
        })
    return in_maps


def kernel(segmentation_logits: np.ndarray, class_gt: np.ndarray) -> np.ndarray:
    segmentation_logits = np.asarray(segmentation_logits, dtype=np.float32)
    class_gt = np.asarray(class_gt, dtype=np.float32)
    B = segmentation_logits.shape[0]
    assert B == N_CORES * IMGS_PER_CORE

    nc = _get_program()
    in_maps = make_in_maps(segmentation_logits, class_gt)
    results = run_bass_kernel_spmd(nc, in_maps, list(range(N_CORES))).results
    total = sum(float(results[c]["partial"][0, 0]) for c in range(N_CORES))
    loss = -total / (B * N_CLASSES)
    return np.float32(loss)


# revision 5
# speedup vs baseline: 1.1686x; 1.1686x over previous
"""Trainium2 Bass kernel for nn_AveragePoolingClassLoss (v2: bf16 + int16 codes).

Reference computation (per image):
  pred = softmax(logits[:, :5], axis=1)            # drop background ch 5
  idx  = argmax_c pred                             # per-pixel class
  s_c  = sum of pred[c] over pixels with idx == c  # == sum of per-pixel max prob
  n_c  = count of pixels with idx == c
  agg  = s_c / n_c (0 if n_c == 0)
  loss = BCE(agg, class_gt), mean over (image, class), log clamp -100

v2 strategy:
  * Inputs are cast to bf16 on the host: halves the HBM DMA (the memory
    roofline) with negligible effect on the final scalar (verified 7e-5).
  * exp() is replaced by the Schraudolph trick: k_c = int16(A*l + B) with
    A = 128/ln2; the int16 bit pattern reinterpreted as bf16 IS approx
    exp(l_c) (piecewise-linear-in-mantissa).  One cheap 4x-mode
    tensor_scalar pass replaces five ACT exp passes per image.  The global
    bias B cancels exactly in m = e_max/T, and argmax on codes == argmax
    on logits (monotone).  End-to-end rel err vs fp32 reference: ~7e-5.
  * max tree + equality masks run on int16 codes (exact integer compare).
  * counts ride free on the mask passes (accum_out), sum_m on the m pass.
  * T = sum_c e_c and the masked sums S_c = sum_p m*g_c go to the PE
    (identity matmuls resp. lhsT=m chunk traces); 1/T via ACT Ln+Exp.
  * class 4 stats by subtraction (sum_m, HW totals).

Sharding: pure data parallel over the batch: 8 cores x 4 images.
Each core emits the partial BCE numerator sum over its 20 (image, class)
pairs; the host sums the 8 partials and scales.
"""

import numpy as np
import ml_dtypes
from contextlib import ExitStack

import concourse.bass as bass
import concourse.bacc as bacc
import concourse.mybir as mybir
import concourse.tile as tile
from concourse import bass_isa, masks
from concourse.bass_utils import run_bass_kernel_spmd

F32 = mybir.dt.float32
BF16 = mybir.dt.bfloat16
I16 = mybir.dt.int16
ALU = mybir.AluOpType
ACTF = mybir.ActivationFunctionType

N_CORES = 8
IMGS_PER_CORE = 4
N_CLASSES = 5
HW = 512 * 512           # pixels per image
P = 128                  # partitions
FD = HW // P             # 2048 free-dim elements per plane
NSTAT = 9                # per image: [S0..S3, sum_m, G0..G3]
LOG_CLAMP = -100.0

# Schraudolph bf16-exp code constants: int16(A*l + B) bitcast bf16 ~ exp(l)
SCHR_A = 128.0 / float(np.log(2.0))
SCHR_B = 16256.0 - 7.335


def _build_program(repeat: int = 1):
    nc = bacc.Bacc(
        "TRN2",
        target_bir_lowering=False,
        debug=False,
        enable_asserts=False,
        num_devices=N_CORES,
    )

    logits = nc.dram_tensor(
        "logits", [IMGS_PER_CORE, N_CLASSES, 512, 512], BF16, kind="ExternalInput"
    )
    gt = nc.dram_tensor("gt", [IMGS_PER_CORE, N_CLASSES], F32, kind="ExternalInput")
    partial = nc.dram_tensor("partial", [1, 1], F32, kind="ExternalOutput")

    with ExitStack() as ctx:
        tc = ctx.enter_context(tile.TileContext(nc))
        _kernel_body(ctx, tc, logits.ap(), gt.ap(), partial.ap(), repeat)

    nc.compile()
    return nc


def _kernel_body(ctx, tc, logits, gt, partial, repeat=1):
    nc = tc.nc

    lpool = ctx.enter_context(tc.tile_pool(name="planes", bufs=2))
    kpool = ctx.enter_context(tc.tile_pool(name="codes", bufs=2))
    wpool = ctx.enter_context(tc.tile_pool(name="work", bufs=2))
    spool = ctx.enter_context(tc.tile_pool(name="stats", bufs=2))
    tpool = ctx.enter_context(tc.tile_pool(name="tpsum", bufs=1, space="PSUM"))
    ppool = ctx.enter_context(tc.tile_pool(name="psumT", bufs=2, space="PSUM"))

    ident = spool.tile([P, P], BF16, tag="ident")
    masks.make_identity(nc, ident[:])

    pools = (lpool, kpool, wpool, tpool, ppool, ident)
    for rep in range(repeat):
        stats = spool.tile([P, IMGS_PER_CORE * NSTAT], F32, tag="stats")
        for i in range(IMGS_PER_CORE):
            _image_pass(tc, pools, stats, logits, i)

    allred = spool.tile([P, IMGS_PER_CORE * NSTAT], F32, tag="allred")
    nc.gpsimd.partition_all_reduce(
        allred[:], stats[:], channels=P, reduce_op=bass_isa.ReduceOp.add
    )

    _bce_tail(ctx, tc, allred, gt, partial)


def _image_pass(tc, pools, stats, logits, i):
    nc = tc.nc
    lpool, kpool, wpool, tpool, ppool, ident = pools
    sb = i * NSTAT
    CH = 512                     # psum/T chunk columns
    TC = 128                     # trace chunk columns

    # ---- DMA the 5 bf16 planes into one wide tile --------------------------
    L = lpool.tile([P, N_CLASSES * FD], BF16, tag="L")
    for c in range(N_CLASSES):
        src = logits[i, c].rearrange("(p a) b -> p (a b)", p=P)
        nc.sync.dma_start(out=L[:, c * FD:(c + 1) * FD], in_=src)

    # ---- Schraudolph codes, spread across ACT/Pool/DVE --------------------
    # (all three engines produce bit-identical int16 conversions)
    K = kpool.tile([P, N_CLASSES * FD], I16, tag="K")
    for c, eng in enumerate(("act", "act", "pool", "pool", "dve")):
        ksl = K[:, c * FD:(c + 1) * FD]
        lsl = L[:, c * FD:(c + 1) * FD]
        if eng == "act":
            nc.scalar.activation(ksl, lsl, ACTF.Copy, scale=SCHR_A, bias=SCHR_B)
        elif eng == "pool":
            nc.gpsimd.tensor_scalar(out=ksl, in0=lsl, scalar1=SCHR_A,
                                    scalar2=SCHR_B, op0=ALU.mult, op1=ALU.add)
        else:
            nc.vector.tensor_scalar(out=ksl, in0=lsl, scalar1=SCHR_A,
                                    scalar2=SCHR_B, op0=ALU.mult, op1=ALU.add)
    K3 = K[:].rearrange("p (c f) -> p c f", c=N_CLASSES)
    Kb = K[:].bitcast(BF16)      # the same bits viewed as bf16 ~ exp(l)

    # ---- max tree on int16 codes ------------------------------------------
    # lvl1: [t01 | t23] in one pass on planes {0,2} vs {1,3}
    t2 = wpool.tile([P, 2 * FD], I16, tag="t2")
    t23d = t2[:].rearrange("p (c f) -> p c f", c=2)
    nc.vector.tensor_tensor(t23d, K3[:, 0:3:2], K3[:, 1:4:2], ALU.max)
    t03 = wpool.tile([P, FD], I16, tag="t03")
    nc.vector.tensor_tensor(t03[:], t2[:, 0:FD], t2[:, FD:2 * FD], ALU.max)
    kmax = wpool.tile([P, FD], I16, tag="kmax")
    nc.vector.tensor_tensor(kmax[:], t03[:], K3[:, 4], ALU.max)

    # ---- masks + counts: g_c = [k_c == kmax], count rides on accum --------
    G = wpool.tile([P, 4 * FD], BF16, tag="G")
    for c in range(4):
        nc.vector.scalar_tensor_tensor(
            out=G[:, c * FD:(c + 1) * FD], in0=K3[:, c], scalar=1.0, in1=kmax[:],
            op0=ALU.mult, op1=ALU.is_equal,
            accum_out=stats[:, sb + 5 + c: sb + 6 + c],
        )

    # ---- T = sum_c e_c per chunk on PE; r = exp(-ln T) on ACT -------------
    r = wpool.tile([P, FD], BF16, tag="r")
    for k in range(FD // CH):
        Tps = ppool.tile([P, CH], F32, tag="Tps")
        for c in range(N_CLASSES):
            nc.tensor.matmul(
                out=Tps[:],
                lhsT=ident[:],
                rhs=Kb[:, c * FD + k * CH: c * FD + (k + 1) * CH],
                start=(c == 0), stop=(c == N_CLASSES - 1),
            )
        lnT = wpool.tile([P, CH], F32, tag="lnT")
        nc.scalar.activation(lnT[:], Tps[:], ACTF.Ln)
        nc.scalar.activation(r[:, k * CH:(k + 1) * CH], lnT[:], ACTF.Exp, scale=-1.0)

    # ---- m = e_max * r (+ sum_m) ------------------------------------------
    kmaxb = kmax[:].bitcast(BF16)
    m = wpool.tile([P, FD], BF16, tag="m")
    nc.vector.scalar_tensor_tensor(
        out=m[:], in0=kmaxb, scalar=1.0, in1=r[:],
        op0=ALU.mult, op1=ALU.mult,
        accum_out=stats[:, sb + 4: sb + 5],
    )

    # ---- masked sums via PE traces: tp_c += m_chunk.T @ g_chunk -----------
    tps = []
    for c in range(4):
        tpc = tpool.tile([P, TC], F32, tag=f"tp{c}")
        tps.append(tpc)
    nk = FD // TC
    for k in range(nk):
        for c in range(4):
            nc.tensor.matmul(
                out=tps[c][:],
                lhsT=m[:, k * TC:(k + 1) * TC],
                rhs=G[:, c * FD + k * TC: c * FD + k * TC + TC],
                start=(k == 0), stop=(k == nk - 1),
            )
    # S_c = trace(tp_c): fused mult-by-identity + accum
    for c in range(4):
        dg = wpool.tile([P, TC], F32, tag="dg")
        nc.vector.scalar_tensor_tensor(
            out=dg[:], in0=tps[c][:], scalar=1.0, in1=ident[:],
            op0=ALU.mult, op1=ALU.mult,
            accum_out=stats[:, sb + c: sb + 1 + c],
        )


def _bce_tail(ctx, tc, allred, gt, partial):
    """Tiny per-core tail on partition 0: build per-(image,class) agg then BCE."""
    nc = tc.nc
    tpool = ctx.enter_context(tc.tile_pool(name="tail", bufs=1))
    NI, NC5 = IMGS_PER_CORE, N_CLASSES
    n20 = NI * NC5

    st = allred[0:1, :]                      # [1, 36]
    st3 = st.rearrange("p (i k) -> p i k", k=NSTAT)  # [1, 4, 9]

    ssum = tpool.tile([1, NI], F32, tag="ssum")
    gsum = tpool.tile([1, NI], F32, tag="gsum")
    nc.vector.reduce_sum(ssum[:], st3[:, :, 0:4], axis=mybir.AxisListType.X)
    nc.vector.reduce_sum(gsum[:], st3[:, :, 5:9], axis=mybir.AxisListType.X)

    A = tpool.tile([1, n20], F32, tag="A")
    C = tpool.tile([1, n20], F32, tag="C")
    A3 = A.rearrange("p (i c) -> p i c", c=NC5)
    C3 = C.rearrange("p (i c) -> p i c", c=NC5)
    nc.vector.tensor_copy(A3[:, :, 0:4], st3[:, :, 0:4])
    nc.vector.tensor_copy(C3[:, :, 0:4], st3[:, :, 5:9])
    nc.vector.tensor_tensor(A3[:, :, 4], st3[:, :, 4], ssum[:], ALU.subtract)
    nc.vector.tensor_scalar(
        out=C3[:, :, 4], in0=gsum[:], scalar1=-1.0, scalar2=float(HW),
        op0=ALU.mult, op1=ALU.add,
    )

    nc.vector.tensor_scalar_max(C[:], C[:], 1.0)
    rc = tpool.tile([1, n20], F32, tag="rc")
    nc.vector.reciprocal(rc[:], C[:])
    agg = tpool.tile([1, n20], F32, tag="agg")
    nc.vector.tensor_tensor(agg[:], A[:], rc[:], ALU.mult)

    logp = tpool.tile([1, n20], F32, tag="logp")
    q = tpool.tile([1, n20], F32, tag="q")
    logq = tpool.tile([1, n20], F32, tag="logq")
    nc.scalar.activation(logp[:], agg[:], ACTF.Ln)
    nc.vector.tensor_scalar_max(logp[:], logp[:], LOG_CLAMP)
    nc.vector.tensor_scalar(
        out=q[:], in0=agg[:], scalar1=-1.0, scalar2=1.0, op0=ALU.mult, op1=ALU.add
    )
    nc.scalar.activation(logq[:], q[:], ACTF.Ln)
    nc.vector.tensor_scalar_max(logq[:], logq[:], LOG_CLAMP)

    gtt = tpool.tile([1, n20], F32, tag="gtt")
    nc.sync.dma_start(out=gtt[:], in_=gt.rearrange("(o i) c -> o (i c)", o=1))
    t1 = tpool.tile([1, n20], F32, tag="t1")
    nc.vector.tensor_tensor(t1[:], gtt[:], logp[:], ALU.mult)
    gtc = tpool.tile([1, n20], F32, tag="gtc")
    nc.vector.tensor_scalar(
        out=gtc[:], in0=gtt[:], scalar1=-1.0, scalar2=1.0, op0=ALU.mult, op1=ALU.add
    )
    t2 = tpool.tile([1, n20], F32, tag="t2")
    nc.vector.tensor_tensor(t2[:], gtc[:], logq[:], ALU.mult)
    tsum = tpool.tile([1, n20], F32, tag="tsum")
    nc.vector.tensor_tensor(tsum[:], t1[:], t2[:], ALU.add)
    out = tpool.tile([1, 1], F32, tag="out")
    nc.vector.reduce_sum(out[:], tsum[:], axis=mybir.AxisListType.X)
    nc.sync.dma_start(out=partial[:], in_=out[:])


_NC_CACHE = {}


def _get_program(repeat: int = 1):
    if repeat not in _NC_CACHE:
        _NC_CACHE[repeat] = _build_program(repeat)
    return _NC_CACHE[repeat]


def make_in_maps(segmentation_logits: np.ndarray, class_gt: np.ndarray):
    seg16 = segmentation_logits[:, :N_CLASSES].astype(ml_dtypes.bfloat16)
    gt32 = np.ascontiguousarray(class_gt, dtype=np.float32)
    in_maps = []
    for core in range(N_CORES):
        lo = core * IMGS_PER_CORE
        hi = lo + IMGS_PER_CORE
        in_maps.append({
            "logits": np.ascontiguousarray(seg16[lo:hi]),
            "gt": np.ascontiguousarray(gt32[lo:hi]),
        })
    return in_maps


def kernel(segmentation_logits: np.ndarray, class_gt: np.ndarray) -> np.ndarray:
    segmentation_logits = np.asarray(segmentation_logits, dtype=np.float32)
    class_gt = np.asarray(class_gt, dtype=np.float32)
    B = segmentation_logits.shape[0]
    assert B == N_CORES * IMGS_PER_CORE

    nc = _get_program()
    in_maps = make_in_maps(segmentation_logits, class_gt)
    results = run_bass_kernel_spmd(nc, in_maps, list(range(N_CORES))).results
    total = sum(float(results[c]["partial"][0, 0]) for c in range(N_CORES))
    loss = -total / (B * N_CLASSES)
    return np.float32(loss)
